# revision 14
# baseline (speedup 1.0000x reference)
"""Causal GQA self-attention (B=4, T=1024, D=2048, H=16, KVH=4, RoPE) on 8 TRN2 cores.

Sharding: 16 (batch, kv-group) units; core c handles batch c//2 and kv-groups
{2*(c%2), 2*(c%2)+1} (= 8 query heads). Wq/Wkv column-sharded, Wo row-sharded
(Megatron attention TP); each core returns a partial [T, D] output and the host
sums the two partials per batch.

v3: all matmuls bf16 (full PE rate at every width, half the DMA traffic).
V is produced as V^T (stationary = Wv tile, 512-wide moving x) so LDWs hide,
then PE-transposed back to natural layout.  K/Q projections are software-
pipelined one unit deep so the RoPE chain (ACT copy -> PE half-swap matmul ->
DVE muls) of unit u hides under unit u+1's projection matmuls.  Attention runs
two head-streams interleaved in PE program order so each stream's ACT exp
hides under the other stream's matmuls; softmax denominators accumulate via
per-item ones-column matmuls into [1,512] psum rows, O^T is copied out of psum
unnormalized (freeing the psum bank immediately) and normalized later by an
in-place DVE multiply with the gpsimd-broadcast 1/l - all off the PE critical
path.  Inputs stream on three DMA queues (sync/scalar: x + y-out, gpsimd:
wk/tables then wq/wo prefetch).
"""

import sys

if "/opt/trn_rl_repo" not in sys.path:
    sys.path.insert(0, "/opt/trn_rl_repo")

from contextlib import ExitStack

import numpy as np

B, T, DIM = 4, 1024, 2048
H, KVH, HD = 16, 4, 128
G = H // KVH
P = 128
KO = DIM // P            # 16 contraction tiles
TT = T // P              # 8 token tiles
HPC = 8                  # heads per core
LG = 2                   # local kv groups per core
QBS = 512                # q block size
NQB = T // QBS           # 2
CBS = 512                # Wo col block size
NCB = DIM // CBS         # 4
SCALE = float(1.0 / np.sqrt(HD))
NCORES = 8

_PROG_CACHE = {}


def _build_program():
    import concourse.bacc as bacc
    import concourse.mybir as mybir
    import concourse.tile as tile

    f32 = mybir.dt.float32
    bf16 = mybir.dt.bfloat16
    EXP = mybir.ActivationFunctionType.Exp

    nc = bacc.Bacc("TRN2", debug=False)

    xt_d = nc.dram_tensor("xt", [P, KO, T], bf16, kind="ExternalInput").ap()
    wq_d = nc.dram_tensor("wq", [P, HPC, KO, P], bf16, kind="ExternalInput").ap()
    wk_d = nc.dram_tensor("wk", [P, KO, LG * HD], bf16, kind="ExternalInput").ap()
    wv_d = nc.dram_tensor("wv", [P, LG, KO, HD], bf16, kind="ExternalInput").ap()
    wo_d = nc.dram_tensor("wo", [P, NCB, HPC, CBS], bf16, kind="ExternalInput").ap()
    cc_d = nc.dram_tensor("cc", [P, T], bf16, kind="ExternalInput").ap()
    nss_d = nc.dram_tensor("nss", [P, T], bf16, kind="ExternalInput").ap()
    tri_d = nc.dram_tensor("tri", [P, P], bf16, kind="ExternalInput").ap()
    swp_d = nc.dram_tensor("swp", [P, P], bf16, kind="ExternalInput").ap()
    eye_d = nc.dram_tensor("eye", [P, P], bf16, kind="ExternalInput").ap()
    y_d = nc.dram_tensor("y", [T, DIM], bf16, kind="ExternalOutput").ap()
    y_r = y_d.rearrange("(to p) c -> p to c", p=P)

    with tile.TileContext(nc) as tc, ExitStack() as ctx:
        const = ctx.enter_context(tc.tile_pool(name="const", bufs=1))
        wfix = ctx.enter_context(tc.tile_pool(name="wfix", bufs=1))
        wqst = ctx.enter_context(tc.tile_pool(name="wqst", bufs=3))
        wop = ctx.enter_context(tc.tile_pool(name="wop", bufs=1))
        xtp = ctx.enter_context(tc.tile_pool(name="xtp", bufs=1))
        big = ctx.enter_context(tc.tile_pool(name="big", bufs=1))
        recp = ctx.enter_context(tc.tile_pool(name="recp", bufs=2))
        rec128p = ctx.enter_context(tc.tile_pool(name="rec128p", bufs=2))
        ptp = ctx.enter_context(tc.tile_pool(name="ptp", bufs=6))
        tmp = ctx.enter_context(tc.tile_pool(name="tmp", bufs=2))
        ysbp = ctx.enter_context(tc.tile_pool(name="ysbp", bufs=4))

        psA = ctx.enter_context(tc.tile_pool(name="psA", bufs=2, space="PSUM"))
        psB = ctx.enter_context(tc.tile_pool(name="psB", bufs=2, space="PSUM"))
        psC = ctx.enter_context(tc.tile_pool(name="psC", bufs=2, space="PSUM"))
        psD = ctx.enter_context(tc.tile_pool(name="psD", bufs=2, space="PSUM"))

        # ---- SBUF tiles ----
        ccsb = const.tile([P, T], bf16, tag="cc", name="cc")
        nsssb = const.tile([P, T], bf16, tag="nss", name="nss")
        trisb = const.tile([P, P], bf16, tag="tri", name="tri")
        swpsb = const.tile([P, P], bf16, tag="swp", name="swp")
        eyesb = const.tile([P, P], bf16, tag="eye", name="eye")
        ones_col = trisb[:, P - 1 : P]

        wvsb = wfix.tile([P, LG, KO, HD], bf16, tag="wv", name="wv")
        wksb = wfix.tile([P, KO, LG * HD], bf16, tag="wk", name="wk")
        xtsb = xtp.tile([P, KO, T], bf16, tag="xt", name="xt")

        vtsb = big.tile([P, LG, T], bf16, tag="vt", name="vt")
        vsb = big.tile([P, TT, LG * HD], bf16, tag="v", name="v")
        ktsb = big.tile([P, LG, T], bf16, tag="kt", name="kt")
        qtsb = [big.tile([P, T], bf16, tag=f"qt{h}", name=f"qt{h}")
                for h in range(HPC)]
        otsb = [big.tile([P, T], bf16, tag=f"ot{h}", name=f"ot{h}")
                for h in range(HPC)]

        # ---- DMA issue ----
        # scalar queue: wv then odd x chunks; sync queue: even x chunks.
        # wq heads 3..7 ride the sync/scalar queues after x (idle by then);
        # their issues block on the wq pool rotation, which is harmless there.
        nc.scalar.dma_start(wvsb[:], wv_d)
        for i in range(KO):
            eng = nc.sync if i % 2 == 0 else nc.scalar
            eng.dma_start(xtsb[:, i : i + 1, :], xt_d[:, i : i + 1, :])
        # gpsimd queue: wk/tables needed by V+K, first three wq heads, wo.
        nc.gpsimd.dma_start(wksb[:], wk_d)
        nc.gpsimd.dma_start(eyesb[:], eye_d)
        nc.gpsimd.dma_start(swpsb[:], swp_d)
        nc.gpsimd.dma_start(ccsb[:], cc_d)
        nc.gpsimd.dma_start(nsssb[:], nss_d)
        nc.gpsimd.dma_start(trisb[:], tri_d)
        wq_tiles = [wqst.tile([P, KO, P], bf16, tag="wq", name=f"wq{lh}")
                    for lh in range(HPC)]
        for lh in range(HPC):
            eng = nc.sync if lh % 2 == 1 else nc.scalar
            eng.dma_start(wq_tiles[lh][:], wq_d[:, lh])
        wosbs = []
        for cb in range(NCB):
            w = wop.tile([P, HPC, CBS], bf16, tag=f"wo{cb}", name=f"wo{cb}")
            nc.gpsimd.dma_start(w[:], wo_d[:, cb])
            wosbs.append(w)

        # ---- V^T projection and K projection interleaved per x chunk so the
        # PE paces with the x DMA as one continuous stream (8 psum banks:
        # 4 V^T accumulators + 4 K accumulators).
        vt_ps = {}
        for lg in range(LG):
            pool = psA if lg == 0 else psB
            for hf in range(NQB):
                vt_ps[(lg, hf)] = pool.tile(
                    [P, QBS], f32, tag=["a", "b"][lg], name=f"vt{lg}{hf}"
                )
        kp_ps = {}
        for lg in range(LG):
            for hf in range(NQB):
                pool = psC if hf == 0 else psD
                kp_ps[(lg, hf)] = pool.tile(
                    [P, QBS], f32, tag=["c", "d"][hf], name=f"kp{lg}{hf}"
                )
        for kt in range(KO):
            for lg in range(LG):
                for hf in range(NQB):
                    nc.tensor.matmul(
                        vt_ps[(lg, hf)][:],
                        wvsb[:, lg, kt, :],
                        xtsb[:, kt, hf * QBS : (hf + 1) * QBS],
                        start=(kt == 0),
                        stop=(kt == KO - 1),
                    )
            for lg in range(LG):
                for hf in range(NQB):
                    nc.tensor.matmul(
                        kp_ps[(lg, hf)][:],
                        wksb[:, kt, lg * HD : (lg + 1) * HD],
                        xtsb[:, kt, hf * QBS : (hf + 1) * QBS],
                        start=(kt == 0),
                        stop=(kt == KO - 1),
                    )

        def rope(src_ps, dst, blk):
            """dst = rope(src_ps) for absolute-t column slice blk.

            The half-swap matmul writes back into src_ps (free once usb is
            copied out), so rope needs no psum tile of its own."""
            usb = tmp.tile([P, QBS], bf16, tag="usb", name="usb")
            nc.scalar.copy(usb[:], src_ps[:])
            nc.tensor.matmul(src_ps[:], swpsb[:], usb[:], start=True, stop=True)
            t1 = tmp.tile([P, QBS], bf16, tag="t1", name="t1")
            nc.vector.tensor_mul(t1[:], usb[:], ccsb[:, blk])
            t2 = tmp.tile([P, QBS], bf16, tag="t2", name="t2")
            nc.vector.tensor_mul(t2[:], src_ps[:], nsssb[:, blk])
            nc.vector.tensor_add(dst, t1[:], t2[:])

        def emit_k_rope(lg):
            for hf in range(NQB):
                blk = slice(hf * QBS, (hf + 1) * QBS)
                rope(kp_ps[(lg, hf)], ktsb[:, lg, blk], blk)

        def emit_q_proj(lh):
            pp = [psC.tile([P, QBS], f32, tag="c", name=f"qp{lh}_0"),
                  psD.tile([P, QBS], f32, tag="d", name=f"qp{lh}_1")]
            for kt in range(KO):
                for hf in range(NQB):
                    nc.tensor.matmul(
                        pp[hf][:],
                        wq_tiles[lh][:, kt, :],
                        xtsb[:, kt, hf * QBS : (hf + 1) * QBS],
                        start=(kt == 0),
                        stop=(kt == KO - 1),
                    )
            return pp

        def emit_q_rope(lh, pp):
            for hf in range(NQB):
                blk = slice(hf * QBS, (hf + 1) * QBS)
                rope(pp[hf], qtsb[lh][:, blk], blk)

        # K ropes right away (frees the kp banks for Q), first Q projection,
        # then the V transposes, then the remaining Q units with their ropes
        # software-pipelined one unit deep.
        emit_k_rope(0)
        emit_k_rope(1)
        qpp = emit_q_proj(0)
        for lg in range(LG):
            for hf in range(NQB):
                nc.scalar.copy(
                    vtsb[:, lg, hf * QBS : (hf + 1) * QBS], vt_ps[(lg, hf)][:]
                )
        for lg in range(LG):
            for tt in range(TT):
                tp = psB.tile([P, P], bf16, tag="b", name="tp")
                nc.tensor.transpose(
                    tp[:], vtsb[:, lg, tt * P : (tt + 1) * P], eyesb[:]
                )
                nc.vector.tensor_copy(
                    out=vsb[:, tt, lg * HD : (lg + 1) * HD], in_=tp[:]
                )
        prev = (0, qpp)
        for lh in range(1, HPC):
            pp = emit_q_proj(lh)
            emit_q_rope(*prev)
            prev = (lh, pp)
        emit_q_rope(*prev)

        # ---- attention: two head-streams interleaved in PE program order so
        # each stream's exp hides under the other stream's matmuls.
        sp_pools = [psA, psB]
        sp_tags = ["a", "b"]

        def attn_pair(lhs, qb):
            items = [(kt, 0, False) for kt in range(4 * qb)]
            items += [(4 * qb + j, P * j, True) for j in range(4)]
            nitems = len(items)
            ops = [psC.tile([P, QBS], f32, tag="c", name=f"op{s}")
                   for s in range(2)]
            lps = [psD.tile([1, QBS], f32, tag="d", name=f"lp{s}")
                   for s in range(2)]
            for idx, (kt, c0, diag) in enumerate(items):
                first = idx == 0
                last = idx == nitems - 1
                for s, lh in enumerate(lhs):
                    lg = lh // 4
                    ncols = QBS - c0
                    sp = sp_pools[s].tile(
                        [P, QBS], f32, tag=sp_tags[s], name=f"sp{s}")
                    nc.tensor.matmul(
                        sp[:, 0:ncols],
                        ktsb[:, lg, kt * P : (kt + 1) * P],
                        qtsb[lh][:, qb * QBS + c0 : (qb + 1) * QBS],
                        start=True,
                        stop=True,
                    )
                    pt = ptp.tile([P, QBS], bf16, tag="pt", name="pt")
                    nc.scalar.activation(
                        pt[:, c0:QBS], sp[:, 0:ncols], EXP, scale=SCALE)
                    if diag:
                        nc.vector.tensor_mul(
                            pt[:, c0 : c0 + P], pt[:, c0 : c0 + P], trisb[:])
                    nc.tensor.matmul(
                        lps[s][:, c0:QBS], ones_col, pt[:, c0:QBS],
                        start=first, stop=last,
                    )
                    nc.tensor.matmul(
                        ops[s][:, c0:QBS],
                        vsb[:, kt, lg * HD : (lg + 1) * HD],
                        pt[:, c0:QBS],
                        start=first,
                        stop=last,
                    )
            dsts = [otsb[lh][:, qb * QBS : (qb + 1) * QBS] for lh in lhs]
            # unnormalized O^T out of psum right away (frees the banks);
            # split across DVE/ACT so neither engine backs up
            nc.vector.tensor_copy(out=dsts[0], in_=ops[0][:])
            nc.scalar.copy(dsts[1], ops[1][:])
            # batch both streams' denominators: one reciprocal + one
            # broadcast per pair instead of two
            lsb = recp.tile([1, 2 * QBS], f32, tag="lsb", name="lsb")
            for s in range(2):
                nc.vector.tensor_copy(
                    out=lsb[0:1, s * QBS : (s + 1) * QBS], in_=lps[s][:])
            rec = recp.tile([1, 2 * QBS], f32, tag="rec", name="rec")
            scr = recp.tile([1, 2 * QBS], f32, tag="scr", name="scr")
            nc.vector.reciprocal_approx_accurate(rec[:], lsb[:], scr[:])
            rec128 = rec128p.tile([P, 2 * QBS], f32, tag="rec128", name="rec128")
            nc.gpsimd.partition_broadcast(rec128[:], rec[:])
            # deferred normalize, in place, off the PE critical path
            for s in range(2):
                nc.vector.tensor_mul(
                    dsts[s], dsts[s], rec128[:, s * QBS : (s + 1) * QBS])

        pair_order = [(0, 1), (2, 3), (4, 5), (6, 7)]
        for qb in range(NQB):
            for lhs in pair_order:
                attn_pair(lhs, qb)

        # ---- output projection: per (tt, lh) one LDW feeds 4 col-block
        # matmuls into 4 psum banks; y DMAs alternate sync/scalar queues.
        yp_pools = [psA, psA, psB, psB]
        yp_tags = ["a", "a", "b", "b"]
        for tt in range(TT):
            yps = [
                yp_pools[cb].tile([P, QBS], f32, tag=yp_tags[cb], name="yp")
                for cb in range(NCB)
            ]
            for lh in range(HPC):
                for cb in range(NCB):
                    nc.tensor.matmul(
                        yps[cb][:, 0:CBS],
                        otsb[lh][:, tt * P : (tt + 1) * P],
                        wosbs[cb][:, lh, :],
                        start=(lh == 0),
                        stop=(lh == HPC - 1),
                    )
            for cb in range(NCB):
                ysb = ysbp.tile([P, CBS], bf16, tag="ysb", name="ysb")
                if cb % 2 == 0:
                    nc.vector.tensor_copy(out=ysb[:], in_=yps[cb][:, 0:CBS])
                else:
                    nc.scalar.copy(ysb[:], yps[cb][:, 0:CBS])
                eng = nc.sync if cb % 2 == 0 else nc.scalar
                eng.dma_start(y_r[:, tt, cb * CBS : (cb + 1) * CBS], ysb[:])

    nc.compile()
    return nc


def _get_program():
    if "prog" not in _PROG_CACHE:
        _PROG_CACHE["prog"] = _build_program()
    return _PROG_CACHE["prog"]


def _prep_core(c, x, Wq, Wkv, Wo, cos, sin):
    import ml_dtypes

    mdt = ml_dtypes.bfloat16
    b = c // 2
    pair = c % 2
    groups = [2 * pair, 2 * pair + 1]
    heads = [g * G + i for g in groups for i in range(G)]

    xT = np.ascontiguousarray(x[b].T)                       # [DIM, T]
    xt_p = np.ascontiguousarray(xT.reshape(KO, P, T).transpose(1, 0, 2))

    wq_cols = np.stack([Wq[:, h * HD : (h + 1) * HD] for h in heads], axis=1)
    wq_p = np.ascontiguousarray(
        wq_cols.reshape(KO, P, HPC, HD).transpose(1, 2, 0, 3)
    )  # [P, lh, kt, c]

    kcols = np.concatenate([Wkv[:, g * HD : (g + 1) * HD] for g in groups], axis=1)
    wk_p = np.ascontiguousarray(kcols.reshape(KO, P, LG * HD).transpose(1, 0, 2))
    vcols = np.stack(
        [Wkv[:, KVH * HD + g * HD : KVH * HD + (g + 1) * HD] for g in groups],
        axis=0,
    )  # [LG, DIM, HD]
    wv_p = np.ascontiguousarray(
        vcols.reshape(LG, KO, P, HD).transpose(2, 0, 1, 3)
    )  # [P, lg, kt, hd]

    worows = np.stack([Wo[h * HD : (h + 1) * HD, :] for h in heads], axis=0)
    wo_p = np.ascontiguousarray(
        worows.reshape(HPC, P, NCB, CBS).transpose(1, 2, 0, 3)
    )  # [P, cb, lh, cc]

    cosT = np.ascontiguousarray(cos.T)                       # [64, T]
    sinT = np.ascontiguousarray(sin.T)
    cc_p = np.ascontiguousarray(np.concatenate([cosT, cosT], axis=0))   # [128, T]
    nss_p = np.ascontiguousarray(np.concatenate([-sinT, sinT], axis=0))
    tri_p = np.triu(np.ones((P, P), dtype=np.float32))
    swp_p = np.roll(np.eye(P, dtype=np.float32), 64, axis=0)  # swp[k,m]=1 iff k=(m+64)%128
    eye_p = np.eye(P, dtype=np.float32)

    return {
        "xt": xt_p.astype(mdt),
        "wq": wq_p.astype(mdt),
        "wk": wk_p.astype(mdt),
        "wv": wv_p.astype(mdt),
        "wo": wo_p.astype(mdt),
        "cc": cc_p.astype(mdt),
        "nss": nss_p.astype(mdt),
        "tri": tri_p.astype(mdt),
        "swp": swp_p.astype(mdt),
        "eye": eye_p.astype(mdt),
    }


def _run(inputs, trace=False, trace_kwargs=None):
    from concourse import bass_utils

    x = np.asarray(inputs["x"], dtype=np.float32)
    Wq = np.asarray(inputs["Wq"], dtype=np.float32)
    Wkv = np.asarray(inputs["Wkv"], dtype=np.float32)
    Wo = np.asarray(inputs["Wo"], dtype=np.float32)
    cos = np.asarray(inputs["cos"], dtype=np.float32)
    sin = np.asarray(inputs["sin"], dtype=np.float32)

    nc = _get_program()
    in_maps = [_prep_core(c, x, Wq, Wkv, Wo, cos, sin) for c in range(NCORES)]
    kwargs = {}
    if trace:
        kwargs["trace"] = True
        if trace_kwargs:
            kwargs.update(trace_kwargs)
    res = bass_utils.run_bass_kernel_spmd(
        nc, in_maps, core_ids=list(range(NCORES)), **kwargs
    )
    outs = [np.asarray(r["y"], dtype=np.float32) for r in res.results]
    y = np.stack([outs[2 * b] + outs[2 * b + 1] for b in range(B)], axis=0)
    return y, res


def kernel(**inputs):
    y, _ = _run(inputs, trace=False)
    return y


# revision 15
# speedup vs baseline: 1.1672x; 1.1672x over previous
"""Causal GQA self-attention (B=4, T=1024, D=2048, H=16, KVH=4, RoPE) on 8 TRN2 cores.

Sharding: 16 (batch, kv-group) units; core c handles batch c//2 and kv-groups
{2*(c%2), 2*(c%2)+1} (= 8 query heads). Wq/Wkv column-sharded, Wo row-sharded
(Megatron attention TP); each core returns a partial [T, D] output and the host
sums the two partials per batch.

v3: all matmuls bf16 (full PE rate at every width, half the DMA traffic).
V is produced as V^T (stationary = Wv tile, 512-wide moving x) so LDWs hide,
then PE-transposed back to natural layout.  K/Q projections are software-
pipelined one unit deep so the RoPE chain (ACT copy -> PE half-swap matmul ->
DVE muls) of unit u hides under unit u+1's projection matmuls.  Attention runs
two head-streams interleaved in PE program order so each stream's ACT exp
hides under the other stream's matmuls; softmax denominators accumulate via
per-item ones-column matmuls into [1,512] psum rows, O^T is copied out of psum
unnormalized (freeing the psum bank immediately) and normalized later by an
in-place DVE multiply with the gpsimd-broadcast 1/l - all off the PE critical
path.  Inputs stream on three DMA queues (sync/scalar: x + y-out, gpsimd:
wk/tables then wq/wo prefetch).
"""

import sys

if "/opt/trn_rl_repo" not in sys.path:
    sys.path.insert(0, "/opt/trn_rl_repo")

from contextlib import ExitStack

import numpy as np

B, T, DIM = 4, 1024, 2048
H, KVH, HD = 16, 4, 128
G = H // KVH
P = 128
KO = DIM // P            # 16 contraction tiles
TT = T // P              # 8 token tiles
HPC = 8                  # heads per core
LG = 2                   # local kv groups per core
QBS = 512                # q block size
NQB = T // QBS           # 2
CBS = 512                # Wo col block size
NCB = DIM // CBS         # 4
SCALE = float(1.0 / np.sqrt(HD))
NCORES = 8

_PROG_CACHE = {}


def _build_program():
    import concourse.bacc as bacc
    import concourse.mybir as mybir
    import concourse.tile as tile

    f32 = mybir.dt.float32
    bf16 = mybir.dt.bfloat16
    EXP = mybir.ActivationFunctionType.Exp

    nc = bacc.Bacc("TRN2", debug=False)

    xt_d = nc.dram_tensor("xt", [P, KO, T], bf16, kind="ExternalInput").ap()
    wq_d = nc.dram_tensor("wq", [P, HPC, KO, P], bf16, kind="ExternalInput").ap()
    wk_d = nc.dram_tensor("wk", [P, KO, LG * HD], bf16, kind="ExternalInput").ap()
    wv_d = nc.dram_tensor("wv", [P, LG, KO, HD], bf16, kind="ExternalInput").ap()
    wo_d = nc.dram_tensor("wo", [P, NCB, HPC, CBS], bf16, kind="ExternalInput").ap()
    cc_d = nc.dram_tensor("cc", [P, T], bf16, kind="ExternalInput").ap()
    nss_d = nc.dram_tensor("nss", [P, T], bf16, kind="ExternalInput").ap()
    tri_d = nc.dram_tensor("tri", [P, P], bf16, kind="ExternalInput").ap()
    swp_d = nc.dram_tensor("swp", [P, P], bf16, kind="ExternalInput").ap()
    eye_d = nc.dram_tensor("eye", [P, P], bf16, kind="ExternalInput").ap()
    y_d = nc.dram_tensor("y", [T, DIM], bf16, kind="ExternalOutput").ap()
    y_r = y_d.rearrange("(to p) c -> p to c", p=P)

    with tile.TileContext(nc) as tc, ExitStack() as ctx:
        const = ctx.enter_context(tc.tile_pool(name="const", bufs=1))
        wfix = ctx.enter_context(tc.tile_pool(name="wfix", bufs=1))
        wqst = ctx.enter_context(tc.tile_pool(name="wqst", bufs=3))
        wop = ctx.enter_context(tc.tile_pool(name="wop", bufs=1))
        xtp = ctx.enter_context(tc.tile_pool(name="xtp", bufs=1))
        big = ctx.enter_context(tc.tile_pool(name="big", bufs=1))
        recp = ctx.enter_context(tc.tile_pool(name="recp", bufs=2))
        rec128p = ctx.enter_context(tc.tile_pool(name="rec128p", bufs=2))
        ptp = ctx.enter_context(tc.tile_pool(name="ptp", bufs=6))
        tmp = ctx.enter_context(tc.tile_pool(name="tmp", bufs=2))
        ysbp = ctx.enter_context(tc.tile_pool(name="ysbp", bufs=4))

        psA = ctx.enter_context(tc.tile_pool(name="psA", bufs=2, space="PSUM"))
        psB = ctx.enter_context(tc.tile_pool(name="psB", bufs=2, space="PSUM"))
        psC = ctx.enter_context(tc.tile_pool(name="psC", bufs=2, space="PSUM"))
        psD = ctx.enter_context(tc.tile_pool(name="psD", bufs=2, space="PSUM"))

        # ---- SBUF tiles ----
        ccsb = const.tile([P, T], bf16, tag="cc", name="cc")
        nsssb = const.tile([P, T], bf16, tag="nss", name="nss")
        trisb = const.tile([P, P], bf16, tag="tri", name="tri")
        swpsb = const.tile([P, P], bf16, tag="swp", name="swp")
        eyesb = const.tile([P, P], bf16, tag="eye", name="eye")
        ones_col = trisb[:, P - 1 : P]

        wvsb = wfix.tile([P, LG, KO, HD], bf16, tag="wv", name="wv")
        wksb = wfix.tile([P, KO, LG * HD], bf16, tag="wk", name="wk")
        xtsb = xtp.tile([P, KO, T], bf16, tag="xt", name="xt")

        vtsb = big.tile([P, LG, T], bf16, tag="vt", name="vt")
        vsb = big.tile([P, TT, LG * HD], bf16, tag="v", name="v")
        ktsb = big.tile([P, LG, T], bf16, tag="kt", name="kt")
        qtsb = [big.tile([P, T], bf16, tag=f"qt{h}", name=f"qt{h}")
                for h in range(HPC)]
        otsb = [big.tile([P, T], bf16, tag=f"ot{h}", name=f"ot{h}")
                for h in range(HPC)]

        # ---- DMA issue ----
        # scalar queue: wv then odd x chunks; sync queue: even x chunks.
        # wq heads 3..7 ride the sync/scalar queues after x (idle by then);
        # their issues block on the wq pool rotation, which is harmless there.
        nc.scalar.dma_start(wvsb[:], wv_d)
        for i in range(KO):
            eng = nc.sync if i % 2 == 0 else nc.scalar
            eng.dma_start(xtsb[:, i : i + 1, :], xt_d[:, i : i + 1, :])
        # gpsimd queue: wk/tables needed by V+K, first three wq heads, wo.
        nc.gpsimd.dma_start(wksb[:], wk_d)
        nc.gpsimd.dma_start(eyesb[:], eye_d)
        nc.gpsimd.dma_start(swpsb[:], swp_d)
        nc.gpsimd.dma_start(ccsb[:], cc_d)
        nc.gpsimd.dma_start(nsssb[:], nss_d)
        wq_tiles = [wqst.tile([P, KO, P], bf16, tag="wq", name=f"wq{lh}")
                    for lh in range(HPC)]
        for lh in range(3):
            nc.gpsimd.dma_start(wq_tiles[lh][:], wq_d[:, lh])
        nc.gpsimd.dma_start(trisb[:], tri_d)
        for lh in range(3, HPC):
            eng = nc.sync if lh % 2 == 1 else nc.scalar
            eng.dma_start(wq_tiles[lh][:], wq_d[:, lh])
        wosbs = []
        for cb in range(NCB):
            w = wop.tile([P, HPC, CBS], bf16, tag=f"wo{cb}", name=f"wo{cb}")
            nc.gpsimd.dma_start(w[:], wo_d[:, cb])
            wosbs.append(w)

        # ---- V^T projection and K projection interleaved per x chunk so the
        # PE paces with the x DMA as one continuous stream (8 psum banks:
        # 4 V^T accumulators + 4 K accumulators).
        vt_ps = {}
        for lg in range(LG):
            pool = psA if lg == 0 else psB
            for hf in range(NQB):
                vt_ps[(lg, hf)] = pool.tile(
                    [P, QBS], f32, tag=["a", "b"][lg], name=f"vt{lg}{hf}"
                )
        kp_ps = {}
        for lg in range(LG):
            for hf in range(NQB):
                pool = psC if hf == 0 else psD
                kp_ps[(lg, hf)] = pool.tile(
                    [P, QBS], f32, tag=["c", "d"][hf], name=f"kp{lg}{hf}"
                )
        for kt in range(KO):
            for lg in range(LG):
                for hf in range(NQB):
                    nc.tensor.matmul(
                        vt_ps[(lg, hf)][:],
                        wvsb[:, lg, kt, :],
                        xtsb[:, kt, hf * QBS : (hf + 1) * QBS],
                        start=(kt == 0),
                        stop=(kt == KO - 1),
                    )
            for lg in range(LG):
                for hf in range(NQB):
                    nc.tensor.matmul(
                        kp_ps[(lg, hf)][:],
                        wksb[:, kt, lg * HD : (lg + 1) * HD],
                        xtsb[:, kt, hf * QBS : (hf + 1) * QBS],
                        start=(kt == 0),
                        stop=(kt == KO - 1),
                    )

        def rope(src_ps, dst, blk):
            """dst = rope(src_ps) for absolute-t column slice blk.

            The half-swap matmul writes back into src_ps (free once usb is
            copied out), so rope needs no psum tile of its own."""
            usb = tmp.tile([P, QBS], bf16, tag="usb", name="usb")
            nc.scalar.copy(usb[:], src_ps[:])
            nc.tensor.matmul(src_ps[:], swpsb[:], usb[:], start=True, stop=True)
            t1 = tmp.tile([P, QBS], bf16, tag="t1", name="t1")
            nc.vector.tensor_mul(t1[:], usb[:], ccsb[:, blk])
            t2 = tmp.tile([P, QBS], bf16, tag="t2", name="t2")
            nc.vector.tensor_mul(t2[:], src_ps[:], nsssb[:, blk])
            nc.vector.tensor_add(dst, t1[:], t2[:])

        def emit_k_rope(lg):
            for hf in range(NQB):
                blk = slice(hf * QBS, (hf + 1) * QBS)
                rope(kp_ps[(lg, hf)], ktsb[:, lg, blk], blk)

        def emit_q_proj(lh):
            pp = [psC.tile([P, QBS], f32, tag="c", name=f"qp{lh}_0"),
                  psD.tile([P, QBS], f32, tag="d", name=f"qp{lh}_1")]
            for kt in range(KO):
                for hf in range(NQB):
                    nc.tensor.matmul(
                        pp[hf][:],
                        wq_tiles[lh][:, kt, :],
                        xtsb[:, kt, hf * QBS : (hf + 1) * QBS],
                        start=(kt == 0),
                        stop=(kt == KO - 1),
                    )
            return pp

        def emit_q_rope(lh, pp):
            for hf in range(NQB):
                blk = slice(hf * QBS, (hf + 1) * QBS)
                rope(pp[hf], qtsb[lh][:, blk], blk)

        # K ropes right away (frees the kp banks for Q), first Q projection,
        # then the V transposes, then the remaining Q units with their ropes
        # software-pipelined one unit deep.
        emit_k_rope(0)
        emit_k_rope(1)
        qpp = emit_q_proj(0)
        for lg in range(LG):
            for hf in range(NQB):
                nc.scalar.copy(
                    vtsb[:, lg, hf * QBS : (hf + 1) * QBS], vt_ps[(lg, hf)][:]
                )
        for lg in range(LG):
            for tt in range(TT):
                tp = psB.tile([P, P], bf16, tag="b", name="tp")
                nc.tensor.transpose(
                    tp[:], vtsb[:, lg, tt * P : (tt + 1) * P], eyesb[:]
                )
                nc.vector.tensor_copy(
                    out=vsb[:, tt, lg * HD : (lg + 1) * HD], in_=tp[:]
                )
        prev = (0, qpp)
        for lh in range(1, HPC):
            pp = emit_q_proj(lh)
            emit_q_rope(*prev)
            prev = (lh, pp)
        emit_q_rope(*prev)

        # ---- attention: two head-streams interleaved in PE program order so
        # each stream's exp hides under the other stream's matmuls.
        sp_pools = [psA, psB]
        sp_tags = ["a", "b"]

        def attn_pair(lhs, qb):
            items = [(kt, 0, False) for kt in range(4 * qb)]
            items += [(4 * qb + j, P * j, True) for j in range(4)]
            nitems = len(items)
            ops = [psC.tile([P, QBS], f32, tag="c", name=f"op{s}")
                   for s in range(2)]
            lps = [psD.tile([1, QBS], f32, tag="d", name=f"lp{s}")
                   for s in range(2)]
            for idx, (kt, c0, diag) in enumerate(items):
                first = idx == 0
                last = idx == nitems - 1
                for s, lh in enumerate(lhs):
                    lg = lh // 4
                    ncols = QBS - c0
                    sp = sp_pools[s].tile(
                        [P, QBS], f32, tag=sp_tags[s], name=f"sp{s}")
                    nc.tensor.matmul(
                        sp[:, 0:ncols],
                        ktsb[:, lg, kt * P : (kt + 1) * P],
                        qtsb[lh][:, qb * QBS + c0 : (qb + 1) * QBS],
                        start=True,
                        stop=True,
                    )
                    pt = ptp.tile([P, QBS], bf16, tag="pt", name="pt")
                    nc.scalar.activation(
                        pt[:, c0:QBS], sp[:, 0:ncols], EXP, scale=SCALE)
                    if diag:
                        nc.vector.tensor_mul(
                            pt[:, c0 : c0 + P], pt[:, c0 : c0 + P], trisb[:])
                    nc.tensor.matmul(
                        lps[s][:, c0:QBS], ones_col, pt[:, c0:QBS],
                        start=first, stop=last,
                    )
                    nc.tensor.matmul(
                        ops[s][:, c0:QBS],
                        vsb[:, kt, lg * HD : (lg + 1) * HD],
                        pt[:, c0:QBS],
                        start=first,
                        stop=last,
                    )
            dsts = [otsb[lh][:, qb * QBS : (qb + 1) * QBS] for lh in lhs]
            # unnormalized O^T out of psum right away (frees the banks);
            # split across DVE/ACT so neither engine backs up
            nc.vector.tensor_copy(out=dsts[0], in_=ops[0][:])
            nc.scalar.copy(dsts[1], ops[1][:])
            # batch both streams' denominators: one reciprocal + one
            # broadcast per pair instead of two
            lsb = recp.tile([1, 2 * QBS], f32, tag="lsb", name="lsb")
            for s in range(2):
                nc.vector.tensor_copy(
                    out=lsb[0:1, s * QBS : (s + 1) * QBS], in_=lps[s][:])
            rec = recp.tile([1, 2 * QBS], f32, tag="rec", name="rec")
            scr = recp.tile([1, 2 * QBS], f32, tag="scr", name="scr")
            nc.vector.reciprocal_approx_accurate(rec[:], lsb[:], scr[:])
            rec128 = rec128p.tile([P, 2 * QBS], f32, tag="rec128", name="rec128")
            nc.gpsimd.partition_broadcast(rec128[:], rec[:])
            # deferred normalize, in place, off the PE critical path
            for s in range(2):
                nc.vector.tensor_mul(
                    dsts[s], dsts[s], rec128[:, s * QBS : (s + 1) * QBS])

        pair_order = [(0, 1), (2, 3), (4, 5), (6, 7)]
        for qb in range(NQB):
            for lhs in pair_order:
                attn_pair(lhs, qb)

        # ---- output projection: per (tt, lh) one LDW feeds 4 col-block
        # matmuls into 4 psum banks; y DMAs alternate sync/scalar queues.
        yp_pools = [psA, psA, psB, psB]
        yp_tags = ["a", "a", "b", "b"]
        for tt in range(TT):
            yps = [
                yp_pools[cb].tile([P, QBS], f32, tag=yp_tags[cb], name="yp")
                for cb in range(NCB)
            ]
            for lh in range(HPC):
                for cb in range(NCB):
                    nc.tensor.matmul(
                        yps[cb][:, 0:CBS],
                        otsb[lh][:, tt * P : (tt + 1) * P],
                        wosbs[cb][:, lh, :],
                        start=(lh == 0),
                        stop=(lh == HPC - 1),
                    )
            for cb in range(NCB):
                ysb = ysbp.tile([P, CBS], bf16, tag="ysb", name="ysb")
                if cb % 2 == 0:
                    nc.vector.tensor_copy(out=ysb[:], in_=yps[cb][:, 0:CBS])
                else:
                    nc.scalar.copy(ysb[:], yps[cb][:, 0:CBS])
                eng = nc.sync if cb % 2 == 0 else nc.scalar
                eng.dma_start(y_r[:, tt, cb * CBS : (cb + 1) * CBS], ysb[:])

    nc.compile()
    return nc


def _get_program():
    if "prog" not in _PROG_CACHE:
        _PROG_CACHE["prog"] = _build_program()
    return _PROG_CACHE["prog"]


def _prep_core(c, x, Wq, Wkv, Wo, cos, sin):
    import ml_dtypes

    mdt = ml_dtypes.bfloat16
    b = c // 2
    pair = c % 2
    groups = [2 * pair, 2 * pair + 1]
    heads = [g * G + i for g in groups for i in range(G)]

    xT = np.ascontiguousarray(x[b].T)                       # [DIM, T]
    xt_p = np.ascontiguousarray(xT.reshape(KO, P, T).transpose(1, 0, 2))

    wq_cols = np.stack([Wq[:, h * HD : (h + 1) * HD] for h in heads], axis=1)
    wq_p = np.ascontiguousarray(
        wq_cols.reshape(KO, P, HPC, HD).transpose(1, 2, 0, 3)
    )  # [P, lh, kt, c]

    kcols = np.concatenate([Wkv[:, g * HD : (g + 1) * HD] for g in groups], axis=1)
    wk_p = np.ascontiguousarray(kcols.reshape(KO, P, LG * HD).transpose(1, 0, 2))
    vcols = np.stack(
        [Wkv[:, KVH * HD + g * HD : KVH * HD + (g + 1) * HD] for g in groups],
        axis=0,
    )  # [LG, DIM, HD]
    wv_p = np.ascontiguousarray(
        vcols.reshape(LG, KO, P, HD).transpose(2, 0, 1, 3)
    )  # [P, lg, kt, hd]

    worows = np.stack([Wo[h * HD : (h + 1) * HD, :] for h in heads], axis=0)
    wo_p = np.ascontiguousarray(
        worows.reshape(HPC, P, NCB, CBS).transpose(1, 2, 0, 3)
    )  # [P, cb, lh, cc]

    cosT = np.ascontiguousarray(cos.T)                       # [64, T]
    sinT = np.ascontiguousarray(sin.T)
    cc_p = np.ascontiguousarray(np.concatenate([cosT, cosT], axis=0))   # [128, T]
    nss_p = np.ascontiguousarray(np.concatenate([-sinT, sinT], axis=0))
    tri_p = np.triu(np.ones((P, P), dtype=np.float32))
    swp_p = np.roll(np.eye(P, dtype=np.float32), 64, axis=0)  # swp[k,m]=1 iff k=(m+64)%128
    eye_p = np.eye(P, dtype=np.float32)

    return {
        "xt": xt_p.astype(mdt),
        "wq": wq_p.astype(mdt),
        "wk": wk_p.astype(mdt),
        "wv": wv_p.astype(mdt),
        "wo": wo_p.astype(mdt),
        "cc": cc_p.astype(mdt),
        "nss": nss_p.astype(mdt),
        "tri": tri_p.astype(mdt),
        "swp": swp_p.astype(mdt),
        "eye": eye_p.astype(mdt),
    }


def _run(inputs, trace=False, trace_kwargs=None):
    from concourse import bass_utils

    x = np.asarray(inputs["x"], dtype=np.float32)
    Wq = np.asarray(inputs["Wq"], dtype=np.float32)
    Wkv = np.asarray(inputs["Wkv"], dtype=np.float32)
    Wo = np.asarray(inputs["Wo"], dtype=np.float32)
    cos = np.asarray(inputs["cos"], dtype=np.float32)
    sin = np.asarray(inputs["sin"], dtype=np.float32)

    nc = _get_program()
    in_maps = [_prep_core(c, x, Wq, Wkv, Wo, cos, sin) for c in range(NCORES)]
    kwargs = {}
    if trace:
        kwargs["trace"] = True
        if trace_kwargs:
            kwargs.update(trace_kwargs)
    res = bass_utils.run_bass_kernel_spmd(
        nc, in_maps, core_ids=list(range(NCORES)), **kwargs
    )
    outs = [np.asarray(r["y"], dtype=np.float32) for r in res.results]
    y = np.stack([outs[2 * b] + outs[2 * b + 1] for b in range(B)], axis=0)
    return y, res


def kernel(**inputs):
    y, _ = _run(inputs, trace=False)
    return y


# revision 17
# speedup vs baseline: 1.1959x; 1.0246x over previous
"""Causal GQA self-attention (B=4, T=1024, D=2048, H=16, KVH=4, RoPE) on 8 TRN2 cores.

Sharding: 16 (batch, kv-group) units; core c handles batch c//2 and kv-groups
{2*(c%2), 2*(c%2)+1} (= 8 query heads). Wq/Wkv column-sharded, Wo row-sharded
(Megatron attention TP); each core returns a partial [T, D] output and the host
sums the two partials per batch.

v3: all matmuls bf16 (full PE rate at every width, half the DMA traffic).
V is produced as V^T (stationary = Wv tile, 512-wide moving x) so LDWs hide,
then PE-transposed back to natural layout.  K/Q projections are software-
pipelined one unit deep so the RoPE chain (ACT copy -> PE half-swap matmul ->
DVE muls) of unit u hides under unit u+1's projection matmuls.  Attention runs
two head-streams interleaved in PE program order so each stream's ACT exp
hides under the other stream's matmuls; softmax denominators accumulate via
per-item ones-column matmuls into [1,512] psum rows, O^T is copied out of psum
unnormalized (freeing the psum bank immediately) and normalized later by an
in-place DVE multiply with the gpsimd-broadcast 1/l - all off the PE critical
path.  Inputs stream on three DMA queues (sync/scalar: x + y-out, gpsimd:
wk/tables then wq/wo prefetch).
"""

import sys

if "/opt/trn_rl_repo" not in sys.path:
    sys.path.insert(0, "/opt/trn_rl_repo")

from contextlib import ExitStack

import numpy as np

B, T, DIM = 4, 1024, 2048
H, KVH, HD = 16, 4, 128
G = H // KVH
P = 128
KO = DIM // P            # 16 contraction tiles
TT = T // P              # 8 token tiles
HPC = 8                  # heads per core
LG = 2                   # local kv groups per core
QBS = 512                # q block size
NQB = T // QBS           # 2
CBS = 512                # Wo col block size
NCB = DIM // CBS         # 4
SCALE = float(1.0 / np.sqrt(HD))
NCORES = 8

_PROG_CACHE = {}


def _build_program():
    import concourse.bacc as bacc
    import concourse.mybir as mybir
    import concourse.tile as tile

    f32 = mybir.dt.float32
    bf16 = mybir.dt.bfloat16
    EXP = mybir.ActivationFunctionType.Exp

    nc = bacc.Bacc("TRN2", debug=False)

    xt_d = nc.dram_tensor("xt", [P, KO, T], bf16, kind="ExternalInput").ap()
    wq_d = nc.dram_tensor("wq", [P, HPC, KO, P], bf16, kind="ExternalInput").ap()
    wk_d = nc.dram_tensor("wk", [P, KO, LG * HD], bf16, kind="ExternalInput").ap()
    wv_d = nc.dram_tensor("wv", [P, LG, KO, HD], bf16, kind="ExternalInput").ap()
    wo_d = nc.dram_tensor("wo", [P, NCB, HPC, CBS], bf16, kind="ExternalInput").ap()
    cc_d = nc.dram_tensor("cc", [P, T], bf16, kind="ExternalInput").ap()
    nss_d = nc.dram_tensor("nss", [P, T], bf16, kind="ExternalInput").ap()
    tri_d = nc.dram_tensor("tri", [P, P], bf16, kind="ExternalInput").ap()
    swp_d = nc.dram_tensor("swp", [P, P], bf16, kind="ExternalInput").ap()
    eye_d = nc.dram_tensor("eye", [P, P], bf16, kind="ExternalInput").ap()
    y_d = nc.dram_tensor("y", [T, DIM], bf16, kind="ExternalOutput").ap()
    y_r = y_d.rearrange("(to p) c -> p to c", p=P)

    with tile.TileContext(nc) as tc, ExitStack() as ctx:
        const = ctx.enter_context(tc.tile_pool(name="const", bufs=1))
        wfix = ctx.enter_context(tc.tile_pool(name="wfix", bufs=1))
        wqst = ctx.enter_context(tc.tile_pool(name="wqst", bufs=3))
        wop = ctx.enter_context(tc.tile_pool(name="wop", bufs=1))
        xtp = ctx.enter_context(tc.tile_pool(name="xtp", bufs=1))
        big = ctx.enter_context(tc.tile_pool(name="big", bufs=1))
        recp = ctx.enter_context(tc.tile_pool(name="recp", bufs=2))
        rec128p = ctx.enter_context(tc.tile_pool(name="rec128p", bufs=2))
        ptp = ctx.enter_context(tc.tile_pool(name="ptp", bufs=6))
        tmp = ctx.enter_context(tc.tile_pool(name="tmp", bufs=2))
        ysbp = ctx.enter_context(tc.tile_pool(name="ysbp", bufs=4))

        psA = ctx.enter_context(tc.tile_pool(name="psA", bufs=2, space="PSUM"))
        psB = ctx.enter_context(tc.tile_pool(name="psB", bufs=2, space="PSUM"))
        psC = ctx.enter_context(tc.tile_pool(name="psC", bufs=2, space="PSUM"))
        psD = ctx.enter_context(tc.tile_pool(name="psD", bufs=2, space="PSUM"))

        # ---- SBUF tiles ----
        ccsb = const.tile([P, T], bf16, tag="cc", name="cc")
        nsssb = const.tile([P, T], bf16, tag="nss", name="nss")
        trisb = const.tile([P, P], bf16, tag="tri", name="tri")
        swpsb = const.tile([P, P], bf16, tag="swp", name="swp")
        eyesb = const.tile([P, P], bf16, tag="eye", name="eye")
        ones_col = trisb[:, P - 1 : P]

        wvsb = wfix.tile([P, LG, KO, HD], bf16, tag="wv", name="wv")
        wksb = wfix.tile([P, KO, LG * HD], bf16, tag="wk", name="wk")
        xtsb = xtp.tile([P, KO, T], bf16, tag="xt", name="xt")

        vtsb = big.tile([P, LG, T], bf16, tag="vt", name="vt")
        vsb = big.tile([P, TT, LG * HD], bf16, tag="v", name="v")
        ktsb = big.tile([P, LG, T], bf16, tag="kt", name="kt")
        qtsb = [big.tile([P, T], bf16, tag=f"qt{h}", name=f"qt{h}")
                for h in range(HPC)]
        otsb = [big.tile([P, T], bf16, tag=f"ot{h}", name=f"ot{h}")
                for h in range(HPC)]

        # ---- DMA issue ----
        # scalar queue: wv then odd x chunks; sync queue: even x chunks.
        # wq heads 3..7 ride the sync/scalar queues after x (idle by then);
        # their issues block on the wq pool rotation, which is harmless there.
        nc.scalar.dma_start(wvsb[:], wv_d)
        for i in range(KO):
            eng = nc.sync if i % 2 == 0 else nc.scalar
            eng.dma_start(xtsb[:, i : i + 1, :], xt_d[:, i : i + 1, :])
        # gpsimd queue: wk/tables needed by V+K, first three wq heads, wo.
        nc.gpsimd.dma_start(wksb[:], wk_d)
        nc.gpsimd.dma_start(eyesb[:], eye_d)
        nc.gpsimd.dma_start(swpsb[:], swp_d)
        nc.gpsimd.dma_start(ccsb[:], cc_d)
        nc.gpsimd.dma_start(nsssb[:], nss_d)
        wq_tiles = [wqst.tile([P, KO, P], bf16, tag="wq", name=f"wq{lh}")
                    for lh in range(HPC)]
        for lh in range(3):
            nc.gpsimd.dma_start(wq_tiles[lh][:], wq_d[:, lh])
        nc.gpsimd.dma_start(trisb[:], tri_d)
        for lh in range(3, HPC):
            eng = nc.sync if lh % 2 == 1 else nc.scalar
            eng.dma_start(wq_tiles[lh][:], wq_d[:, lh])
        wosbs = []
        for cb in range(NCB):
            w = wop.tile([P, HPC, CBS], bf16, tag=f"wo{cb}", name=f"wo{cb}")
            nc.gpsimd.dma_start(w[:], wo_d[:, cb])
            wosbs.append(w)

        # ---- V^T projection and K projection interleaved per x chunk so the
        # PE paces with the x DMA as one continuous stream (8 psum banks:
        # 4 V^T accumulators + 4 K accumulators).
        vt_ps = {}
        for lg in range(LG):
            pool = psA if lg == 0 else psB
            for hf in range(NQB):
                vt_ps[(lg, hf)] = pool.tile(
                    [P, QBS], f32, tag=["a", "b"][lg], name=f"vt{lg}{hf}"
                )
        kp_ps = {}
        for lg in range(LG):
            for hf in range(NQB):
                pool = psC if hf == 0 else psD
                kp_ps[(lg, hf)] = pool.tile(
                    [P, QBS], f32, tag=["c", "d"][hf], name=f"kp{lg}{hf}"
                )
        for kt in range(KO):
            for lg in range(LG):
                for hf in range(NQB):
                    nc.tensor.matmul(
                        vt_ps[(lg, hf)][:],
                        wvsb[:, lg, kt, :],
                        xtsb[:, kt, hf * QBS : (hf + 1) * QBS],
                        start=(kt == 0),
                        stop=(kt == KO - 1),
                    )
            for lg in range(LG):
                for hf in range(NQB):
                    nc.tensor.matmul(
                        kp_ps[(lg, hf)][:],
                        wksb[:, kt, lg * HD : (lg + 1) * HD],
                        xtsb[:, kt, hf * QBS : (hf + 1) * QBS],
                        start=(kt == 0),
                        stop=(kt == KO - 1),
                    )

        def rope(src_ps, dst, blk):
            """dst = rope(src_ps) for absolute-t column slice blk.

            The half-swap matmul writes back into src_ps (free once usb is
            copied out), so rope needs no psum tile of its own."""
            usb = tmp.tile([P, QBS], bf16, tag="usb", name="usb")
            nc.scalar.copy(usb[:], src_ps[:])
            nc.tensor.matmul(src_ps[:], swpsb[:], usb[:], start=True, stop=True)
            t1 = tmp.tile([P, QBS], bf16, tag="t1", name="t1")
            nc.vector.tensor_mul(t1[:], usb[:], ccsb[:, blk])
            t2 = tmp.tile([P, QBS], bf16, tag="t2", name="t2")
            nc.vector.tensor_mul(t2[:], src_ps[:], nsssb[:, blk])
            nc.vector.tensor_add(dst, t1[:], t2[:])

        def emit_k_rope(lg):
            for hf in range(NQB):
                blk = slice(hf * QBS, (hf + 1) * QBS)
                rope(kp_ps[(lg, hf)], ktsb[:, lg, blk], blk)

        def emit_q_proj(lh):
            pp = [psC.tile([P, QBS], f32, tag="c", name=f"qp{lh}_0"),
                  psD.tile([P, QBS], f32, tag="d", name=f"qp{lh}_1")]
            for kt in range(KO):
                for hf in range(NQB):
                    nc.tensor.matmul(
                        pp[hf][:],
                        wq_tiles[lh][:, kt, :],
                        xtsb[:, kt, hf * QBS : (hf + 1) * QBS],
                        start=(kt == 0),
                        stop=(kt == KO - 1),
                    )
            return pp

        def emit_q_rope(lh, pp):
            for hf in range(NQB):
                blk = slice(hf * QBS, (hf + 1) * QBS)
                rope(pp[hf], qtsb[lh][:, blk], blk)

        # K ropes right away (frees the kp banks for Q), first Q projection,
        # then the V transposes, then the remaining Q units with their ropes
        # software-pipelined one unit deep.
        emit_k_rope(0)
        emit_k_rope(1)
        qpp = emit_q_proj(0)
        for lg in range(LG):
            for hf in range(NQB):
                nc.scalar.copy(
                    vtsb[:, lg, hf * QBS : (hf + 1) * QBS], vt_ps[(lg, hf)][:]
                )
        for lg in range(LG):
            for tt in range(TT):
                tp = psB.tile([P, P], bf16, tag="b", name="tp")
                nc.tensor.transpose(
                    tp[:], vtsb[:, lg, tt * P : (tt + 1) * P], eyesb[:]
                )
                nc.vector.tensor_copy(
                    out=vsb[:, tt, lg * HD : (lg + 1) * HD], in_=tp[:]
                )
        prev = (0, qpp)
        for lh in range(1, HPC):
            pp = emit_q_proj(lh)
            emit_q_rope(*prev)
            prev = (lh, pp)
        emit_q_rope(*prev)

        # ---- attention: two head-streams interleaved in PE program order so
        # each stream's exp hides under the other stream's matmuls.
        sp_pools = [psA, psB]
        sp_tags = ["a", "b"]

        def attn_items(qb):
            items = [(kt, 0, False) for kt in range(4 * qb)]
            items += [(4 * qb + j, P * j, True) for j in range(4)]
            return items

        def attn_pair(lhs, qbs):
            """Two interleaved attention rounds: stream s computes head
            lhs[s], query block qbs[s].  Pairing a 4-item qb0 round with an
            8-item qb1 round gives the pair enough PE work to hide the
            DVE masks and ACT exps of both."""
            sitems = [attn_items(qb) for qb in qbs]
            ops = [psC.tile([P, QBS], f32, tag="c", name=f"op{s}")
                   for s in range(2)]
            lps = [psD.tile([1, QBS], f32, tag="d", name=f"lp{s}")
                   for s in range(2)]

            def emit_item(s, idx):
                lh, qb = lhs[s], qbs[s]
                kt, c0, diag = sitems[s][idx]
                first = idx == 0
                last = idx == len(sitems[s]) - 1
                lg = lh // 4
                ncols = QBS - c0
                sp = sp_pools[s].tile(
                    [P, QBS], f32, tag=sp_tags[s], name=f"sp{s}")
                nc.tensor.matmul(
                    sp[:, 0:ncols],
                    ktsb[:, lg, kt * P : (kt + 1) * P],
                    qtsb[lh][:, qb * QBS + c0 : (qb + 1) * QBS],
                    start=True,
                    stop=True,
                )
                pt = ptp.tile([P, QBS], bf16, tag="pt", name="pt")
                nc.scalar.activation(
                    pt[:, c0:QBS], sp[:, 0:ncols], EXP, scale=SCALE)
                if diag:
                    nc.vector.tensor_mul(
                        pt[:, c0 : c0 + P], pt[:, c0 : c0 + P], trisb[:])
                nc.tensor.matmul(
                    lps[s][:, c0:QBS], ones_col, pt[:, c0:QBS],
                    start=first, stop=last,
                )
                nc.tensor.matmul(
                    ops[s][:, c0:QBS],
                    vsb[:, kt, lg * HD : (lg + 1) * HD],
                    pt[:, c0:QBS],
                    start=first,
                    stop=last,
                )

            n0, n1 = len(sitems[0]), len(sitems[1])
            if n0 == n1:
                for idx in range(n0):
                    emit_item(0, idx)
                    emit_item(1, idx)
            else:
                # stream 1 has 2x the items: emit B,A,B per A item
                assert n1 == 2 * n0
                for idx in range(n0):
                    emit_item(1, 2 * idx)
                    emit_item(0, idx)
                    emit_item(1, 2 * idx + 1)
            dsts = [otsb[lhs[s]][:, qbs[s] * QBS : (qbs[s] + 1) * QBS]
                    for s in range(2)]
            # unnormalized O^T out of psum right away (frees the banks);
            # split across DVE/ACT so neither engine backs up
            nc.vector.tensor_copy(out=dsts[0], in_=ops[0][:])
            nc.scalar.copy(dsts[1], ops[1][:])
            # batch both streams' denominators: one reciprocal + one
            # broadcast per pair instead of two
            lsb = recp.tile([1, 2 * QBS], f32, tag="lsb", name="lsb")
            for s in range(2):
                nc.vector.tensor_copy(
                    out=lsb[0:1, s * QBS : (s + 1) * QBS], in_=lps[s][:])
            rec = recp.tile([1, 2 * QBS], f32, tag="rec", name="rec")
            scr = recp.tile([1, 2 * QBS], f32, tag="scr", name="scr")
            nc.vector.reciprocal_approx_accurate(rec[:], lsb[:], scr[:])
            rec128 = rec128p.tile([P, 2 * QBS], f32, tag="rec128", name="rec128")
            nc.gpsimd.partition_broadcast(rec128[:], rec[:])
            # deferred normalize, in place, off the PE critical path
            for s in range(2):
                nc.vector.tensor_mul(
                    dsts[s], dsts[s], rec128[:, s * QBS : (s + 1) * QBS])

        # mixed pairs: head lh's 4-item qb0 round with head lh's 8-item qb1
        # round; all qb0 output is ready well before the Wo phase needs it.
        for lh in range(HPC):
            attn_pair((lh, lh), (0, 1))

        # ---- output projection: per (tt, lh) one LDW feeds 4 col-block
        # matmuls into 4 psum banks; y DMAs alternate sync/scalar queues.
        yp_pools = [psA, psA, psB, psB]
        yp_tags = ["a", "a", "b", "b"]
        for tt in range(TT):
            yps = [
                yp_pools[cb].tile([P, QBS], f32, tag=yp_tags[cb], name="yp")
                for cb in range(NCB)
            ]
            for lh in range(HPC):
                for cb in range(NCB):
                    nc.tensor.matmul(
                        yps[cb][:, 0:CBS],
                        otsb[lh][:, tt * P : (tt + 1) * P],
                        wosbs[cb][:, lh, :],
                        start=(lh == 0),
                        stop=(lh == HPC - 1),
                    )
            for cb in range(NCB):
                ysb = ysbp.tile([P, CBS], bf16, tag="ysb", name="ysb")
                if cb % 2 == 0:
                    nc.vector.tensor_copy(out=ysb[:], in_=yps[cb][:, 0:CBS])
                else:
                    nc.scalar.copy(ysb[:], yps[cb][:, 0:CBS])
                eng = nc.sync if cb % 2 == 0 else nc.scalar
                eng.dma_start(y_r[:, tt, cb * CBS : (cb + 1) * CBS], ysb[:])

    nc.compile()
    return nc


def _get_program():
    if "prog" not in _PROG_CACHE:
        _PROG_CACHE["prog"] = _build_program()
    return _PROG_CACHE["prog"]


def _prep_core(c, x, Wq, Wkv, Wo, cos, sin):
    import ml_dtypes

    mdt = ml_dtypes.bfloat16
    b = c // 2
    pair = c % 2
    groups = [2 * pair, 2 * pair + 1]
    heads = [g * G + i for g in groups for i in range(G)]

    xT = np.ascontiguousarray(x[b].T)                       # [DIM, T]
    xt_p = np.ascontiguousarray(xT.reshape(KO, P, T).transpose(1, 0, 2))

    wq_cols = np.stack([Wq[:, h * HD : (h + 1) * HD] for h in heads], axis=1)
    wq_p = np.ascontiguousarray(
        wq_cols.reshape(KO, P, HPC, HD).transpose(1, 2, 0, 3)
    )  # [P, lh, kt, c]

    kcols = np.concatenate([Wkv[:, g * HD : (g + 1) * HD] for g in groups], axis=1)
    wk_p = np.ascontiguousarray(kcols.reshape(KO, P, LG * HD).transpose(1, 0, 2))
    vcols = np.stack(
        [Wkv[:, KVH * HD + g * HD : KVH * HD + (g + 1) * HD] for g in groups],
        axis=0,
    )  # [LG, DIM, HD]
    wv_p = np.ascontiguousarray(
        vcols.reshape(LG, KO, P, HD).transpose(2, 0, 1, 3)
    )  # [P, lg, kt, hd]

    worows = np.stack([Wo[h * HD : (h + 1) * HD, :] for h in heads], axis=0)
    wo_p = np.ascontiguousarray(
        worows.reshape(HPC, P, NCB, CBS).transpose(1, 2, 0, 3)
    )  # [P, cb, lh, cc]

    cosT = np.ascontiguousarray(cos.T)                       # [64, T]
    sinT = np.ascontiguousarray(sin.T)
    cc_p = np.ascontiguousarray(np.concatenate([cosT, cosT], axis=0))   # [128, T]
    nss_p = np.ascontiguousarray(np.concatenate([-sinT, sinT], axis=0))
    tri_p = np.triu(np.ones((P, P), dtype=np.float32))
    swp_p = np.roll(np.eye(P, dtype=np.float32), 64, axis=0)  # swp[k,m]=1 iff k=(m+64)%128
    eye_p = np.eye(P, dtype=np.float32)

    return {
        "xt": xt_p.astype(mdt),
        "wq": wq_p.astype(mdt),
        "wk": wk_p.astype(mdt),
        "wv": wv_p.astype(mdt),
        "wo": wo_p.astype(mdt),
        "cc": cc_p.astype(mdt),
        "nss": nss_p.astype(mdt),
        "tri": tri_p.astype(mdt),
        "swp": swp_p.astype(mdt),
        "eye": eye_p.astype(mdt),
    }


def _run(inputs, trace=False, trace_kwargs=None):
    from concourse import bass_utils

    x = np.asarray(inputs["x"], dtype=np.float32)
    Wq = np.asarray(inputs["Wq"], dtype=np.float32)
    Wkv = np.asarray(inputs["Wkv"], dtype=np.float32)
    Wo = np.asarray(inputs["Wo"], dtype=np.float32)
    cos = np.asarray(inputs["cos"], dtype=np.float32)
    sin = np.asarray(inputs["sin"], dtype=np.float32)

    nc = _get_program()
    in_maps = [_prep_core(c, x, Wq, Wkv, Wo, cos, sin) for c in range(NCORES)]
    kwargs = {}
    if trace:
        kwargs["trace"] = True
        if trace_kwargs:
            kwargs.update(trace_kwargs)
    res = bass_utils.run_bass_kernel_spmd(
        nc, in_maps, core_ids=list(range(NCORES)), **kwargs
    )
    outs = [np.asarray(r["y"], dtype=np.float32) for r in res.results]
    y = np.stack([outs[2 * b] + outs[2 * b + 1] for b in range(B)], axis=0)
    return y, res


def kernel(**inputs):
    y, _ = _run(inputs, trace=False)
    return y
